# revision 10
# baseline (speedup 1.0000x reference)
"""MCR loss kernel for Trainium2 (8 NeuronCores), v2.

Strategy (per core: 2 timesteps x 3 feature maps = 6 input planes):
  - DMA each plane [32c, 192h, 192w] into SBUF with partitions =
    (c, a) where a = h row within a half h-group; 3 KB contiguous runs.
  - h-direction 8:1 pool on the PE: ones-matrix stationary (loaded per
    matmul, float32r so 384-col moving operands stream at 1 col/cycle),
    3 col-strips = the 3 planes of one timestep, so pooled data lands in
    PSUM as [(m, c), 2hg x 192w].
  - w-direction 8:1 pool on DVE: tensor_reduce PSUM -> SBUF directly
    into a reflect-padded [96, 26x26] conv input (strided dst AP); 4
    small edge copies finish the reflect pad.
  - 3x3 conv as 9 shifted matmuls with block-diag [96,96] stationaries
    (no replication copies); LeakyReLU(0.2) as a single Act-engine
    Lrelu op PSUM->SBUF with bf16 output.
  - Gram G_t = V_t V_t^T via bf16 PE transpose + matmul chunks.
  - Host: logdet(I_576 + a V^T V) = logdet(I_96 + a V V^T); float64
    Cholesky on [16,96,96] Grams finishes the scalar loss.
"""

import numpy as np

_STATE = {}

# -------- fixed problem geometry (hardcoded per harness contract) --------
B, CCH, H, W = 16, 32, 192, 192
NCORES = 8
TPC = B // NCORES          # timesteps per core = 2
OUT = 24                   # pooled spatial size
PIX = OUT * OUT            # 576
M = 96                     # feature rows (3 maps x 32 channels)
ALPHA_E = 6.0              # 576 / (96 * eps)
ALPHA_C = 18.0             # 576 / (32 * eps)
PAD = 26                   # padded conv input edge
PPIX = PAD * PAD           # 676


def _build_nc():
    import concourse.bass as bass
    import concourse.tile as tile
    from concourse import bacc, mybir

    DT = mybir.dt.float32
    BF = mybir.dt.bfloat16
    F32R = mybir.dt.float32r
    ACT = mybir.ActivationFunctionType

    nc = bacc.Bacc(
        "TRN2", target_bir_lowering=False, debug=False, num_devices=NCORES
    )

    # x[g] for g = t*3+m : feature-map plane stacks, host-reordered
    x = nc.declare_dram_parameter("x", [TPC * 3, CCH, H, W], DT, isOutput=False)
    # block-diag conv weights: wt[(m,ic), (dy*3+dx)*96 + (m,oc)]
    wt = nc.declare_dram_parameter("wt", [96, 9 * 96], DT, isOutput=False)
    # pool map: ones[(c*4+a), 32*m + c] = 1
    ones = nc.declare_dram_parameter("ones", [128, 96], BF, isOutput=False)
    ident = nc.declare_dram_parameter("ident", [96, 96], BF, isOutput=False)
    g_out = nc.declare_dram_parameter("g_out", [TPC, M, M], DT, isOutput=True)

    with tile.TileContext(nc) as tc:
        with (
            tc.tile_pool(name="persist", bufs=1) as persist,
            tc.tile_pool(name="planes", bufs=2) as planes,
            tc.tile_pool(name="planesbf", bufs=4) as planesbf,
            tc.tile_pool(name="vt", bufs=2) as vtpool,
            tc.tile_pool(name="poolps", bufs=3, space="PSUM") as poolps,
            tc.tile_pool(name="convps", bufs=2, space="PSUM") as convps,
            tc.tile_pool(name="vtps", bufs=2, space="PSUM") as vtps,
            tc.tile_pool(name="gramps", bufs=1, space="PSUM") as gramps,
        ):
            wt_sb = persist.tile([96, 9 * 96], DT, tag="wt")
            nc.gpsimd.dma_start(out=wt_sb[:], in_=wt.ap())
            ones_sb = persist.tile([128, 96], BF, tag="ones")
            nc.gpsimd.dma_start(out=ones_sb[:], in_=ones.ap())
            id_sb = persist.tile([96, 96], BF, tag="ident")
            nc.gpsimd.dma_start(out=id_sb[:], in_=ident.ap())

            # padded pooled conv inputs, one per timestep; V in bf16
            pad_sb = persist.tile([96, TPC * PPIX], DT, tag="pad")
            v_sb = persist.tile([96, TPC * PIX], BF, tag="v")
            g_sb = persist.tile([96, TPC * 96], DT, tag="g")

            for t in range(TPC):
                # ---- load the 3 planes of this timestep ----
                bufs = []
                for m in range(3):
                    g = t * 3 + m
                    buf = planes.tile([128, 2 * 24 * 192], DT, tag="plane")
                    # src partitions (c, a), free (hg, w); one DMA per
                    # a-half so the AP stays 4D. 3 KB contiguous runs.
                    # partitions are a-major: p = a*32 + c
                    src = x.ap()[g].rearrange(
                        "c (hg hf a) w -> hf a c hg w", hg=24, hf=2, a=4
                    )
                    for hf in range(2):
                        for a in range(4):
                            nc.sync.dma_start(
                                out=buf[
                                    32 * a : 32 * a + 32,
                                    hf * 4608 : (hf + 1) * 4608,
                                ].rearrange("c (hg w) -> c hg w", hg=24),
                                in_=src[hf][a],
                            )
                    bufb = planesbf.tile([128, 2 * 24 * 192], BF, tag="planebf")
                    nc.scalar.copy(bufb[:], buf[:])
                    bufs.append(bufb)

                pad_t = pad_sb[:, t * PPIX : (t + 1) * PPIX]
                pad3 = pad_t.rearrange("p (y x) -> p y x", y=PAD)

                # ---- h-pool on PE + w-pool on DVE, 2 hg rows per tile ----
                for k in range(12):
                    ps = poolps.tile([96, 384], DT, tag="poolps")
                    for m in range(3):
                        b3 = bufs[m][:].rearrange(
                            "p (hf hg w) -> p hf hg w", hf=2, hg=24
                        )
                        for hf in range(2):
                            nc.tensor.matmul(
                                ps[32 * m : 32 * m + 32, :],
                                ones_sb[:, 32 * m : 32 * m + 32],
                                b3[:, hf, 2 * k : 2 * k + 2, :],
                                start=(hf == 0),
                                stop=(hf == 1),
                            )
                    # [96,(hg2,x24,b8)] -> padded rows 2k+1, 2k+2, cols 1..24
                    nc.vector.tensor_reduce(
                        out=pad3[:, 2 * k + 1 : 2 * k + 3, 1:25],
                        in_=ps[:].rearrange("p (hg x b) -> p hg x b", hg=2, b=8),
                        axis=mybir.AxisListType.X,
                        op=mybir.AluOpType.add,
                    )

                # ---- finish reflect pad: rows then cols (corners) ----
                nc.vector.tensor_copy(pad3[:, 0:1, 1:25], pad3[:, 2:3, 1:25])
                nc.vector.tensor_copy(pad3[:, 25:26, 1:25], pad3[:, 23:24, 1:25])
                nc.vector.tensor_copy(pad3[:, :, 0:1], pad3[:, :, 2:3])
                nc.vector.tensor_copy(pad3[:, :, 25:26], pad3[:, :, 23:24])

                # ---- conv: 9 shifted matmuls, block-diag stationary ----
                for yh in range(2):
                    pc = convps.tile([96, 288], DT, tag="convps")
                    for i, (dy, dx) in enumerate(
                        [(a, b) for a in range(3) for b in range(3)]
                    ):
                        blk = dy * 3 + dx
                        nc.tensor.matmul(
                            pc[:],
                            wt_sb[:, blk * 96 : (blk + 1) * 96],
                            pad3[
                                :, yh * 12 + dy : yh * 12 + dy + 12, dx : dx + 24
                            ],
                            start=(i == 0),
                            stop=(i == 8),
                        )
                    # LeakyReLU(0.2) == max(0.2*z, z); PSUM may feed only one
                    # non-scalar input, so stage a copy through SBUF first
                    zc = vtpool.tile([96, 288], DT, tag="zcopy")
                    nc.scalar.copy(zc[:], pc[:])
                    nc.vector.scalar_tensor_tensor(
                        out=v_sb[:, t * PIX + yh * 288 : t * PIX + (yh + 1) * 288],
                        in0=zc[:],
                        scalar=0.2,
                        in1=pc[:],
                        op0=mybir.AluOpType.mult,
                        op1=mybir.AluOpType.max,
                    )

                # ---- Gram: G_t += VT_chunk^T @ VT_chunk (bf16) ----
                gp = gramps.tile([96, 96], DT, tag="gram")
                for c in range(5):
                    sz = 128 if c < 4 else 64
                    vslice = v_sb[:, t * PIX + c * 128 : t * PIX + c * 128 + sz]
                    pt = vtps.tile([128, 96], BF, tag="vtps")
                    nc.tensor.transpose(pt[:sz, :], vslice, id_sb[:])
                    vt = vtpool.tile([128, 96], BF, tag="vt")
                    nc.scalar.copy(vt[:sz, :], pt[:sz, :])
                    nc.tensor.matmul(
                        gp[:], vt[:sz, :], vt[:sz, :],
                        start=(c == 0), stop=(c == 4),
                    )
                nc.scalar.copy(g_sb[:, t * 96 : (t + 1) * 96], gp[:])
                nc.gpsimd.dma_start(
                    out=g_out[t], in_=g_sb[:, t * 96 : (t + 1) * 96]
                )

    nc.finalize()
    return nc


def _get_nc():
    if "nc" not in _STATE:
        _STATE["nc"] = _build_nc()
    return _STATE["nc"]


def _prep_weights(W1, W2, W3):
    # wt[(m,ic), (dy*3+dx)*96 + 32m+oc] = W_m[oc, ic, dy, dx] / 64
    wt = np.zeros((96, 9 * 96), dtype=np.float64)
    for m, Wm in enumerate((W1, W2, W3)):
        Wm = np.asarray(Wm, np.float64) / 64.0  # [oc, ic, dy, dx]
        for dy in range(3):
            for dx in range(3):
                blk = dy * 3 + dx
                # lhsT[ic, oc] block at rows 32m, cols blk*96 + 32m
                wt[
                    32 * m : 32 * m + 32,
                    blk * 96 + 32 * m : blk * 96 + 32 * m + 32,
                ] = Wm[:, :, dy, dx].T
    return np.ascontiguousarray(wt, dtype=np.float32)


def _prep_ones():
    # partitions a-major: p = a*32 + c
    ones = np.zeros((128, 96), dtype=np.float32)
    for c in range(32):
        for a in range(4):
            for m in range(3):
                ones[a * 32 + c, 32 * m + c] = 1.0
    return ones


def _host_loss(G):
    G = np.asarray(G, np.float64)  # [16, 96, 96]
    T = G.shape[0]
    I96 = np.eye(M)
    Me = I96[None] + ALPHA_E * G
    ld_e = 2.0 * np.log(
        np.diagonal(np.linalg.cholesky(Me), axis1=-2, axis2=-1)
    ).sum()
    blocks = np.stack(
        [G[:, 32 * c : 32 * (c + 1), 32 * c : 32 * (c + 1)] for c in range(3)]
    )  # [3, T, 32, 32]
    Mc = np.eye(32)[None, None] + ALPHA_C * blocks
    ld_c = 2.0 * np.log(
        np.diagonal(np.linalg.cholesky(Mc), axis1=-2, axis2=-1)
    ).sum()
    loss_expd = ld_e / (2.0 * T)
    loss_comp = (32.0 / M) * ld_c / (2.0 * T)
    return np.float32(loss_expd - loss_comp)


def run_device(inputs, **kw):
    """Run the bass kernel; returns (G [16,96,96], BassKernelResults)."""
    from concourse.bass_utils import run_bass_kernel_spmd

    nc = _get_nc()
    wt = _prep_weights(inputs["W1"], inputs["W2"], inputs["W3"])
    import ml_dtypes

    ones = _prep_ones().astype(ml_dtypes.bfloat16)
    ident_bf = np.eye(96, dtype=np.float32).astype(ml_dtypes.bfloat16)
    ms = np.asarray(inputs["ms_fea"], np.float32)
    pan = np.asarray(inputs["pan_fea"], np.float32)
    alf = np.asarray(inputs["all_fea"], np.float32)
    in_maps = []
    for i in range(NCORES):
        sl = slice(TPC * i, TPC * (i + 1))
        # x[t*3+m] = (ms,pan,alf)[m][t]
        xs = np.stack([ms[sl], pan[sl], alf[sl]], axis=1).reshape(
            TPC * 3, CCH, H, W
        )
        in_maps.append(
            {
                "x": np.ascontiguousarray(xs),
                "wt": wt,
                "ones": ones,
                "ident": ident_bf,
            }
        )
    res = run_bass_kernel_spmd(nc, in_maps, core_ids=list(range(NCORES)), **kw)
    G = np.concatenate([np.asarray(r["g_out"]) for r in res.results], axis=0)
    return G, res


def kernel(**inputs):
    G, _ = run_device(inputs)
    return _host_loss(G)


# revision 15
# speedup vs baseline: 1.6269x; 1.6269x over previous
"""MCR loss kernel for Trainium2 (8 NeuronCores), v3.

Per core: 2 timesteps x 3 feature maps = 6 input planes [32c, 192h, 192w].

  - DMA: 24-row h-slabs, pass A = planes 0-3 on 128 partitions (g,c),
    pass B = planes 4-5 on 64 partitions; 18.4 KB contiguous runs per
    partition, interleaved A/B so the DVE is continuously fed.
  - stage 1 (w-direction 8:1 pool) on DVE: tensor_reduce over the
    contiguous innermost 8, writing an x-major transposed intermediate
    [p, (x24, h24)] so stage 2 also reduces a contiguous axis.
  - stage 2 (h-direction 8:1) on DVE: reduce over r8 (contiguous),
    writing bf16 directly into a reflect-padded x-major conv input
    [p, 26x, 26y]; 4 small edge copies per pad finish the pad.
  - conv: 9 shifted bf16 matmuls (1 cyc/col) with block-diag [96,96]
    stationaries for t0; 27 row/col-tiled matmuls for t1 (its planes
    straddle the two pad buffers). LeakyReLU(0.2) = Act copy + DVE
    max(0.2z, z), output V in bf16 (x-major pixel order; the Gram is
    invariant to pixel order).
  - Gram G_t = V_t V_t^T via bf16 PE transpose + matmul chunks.
  - Host: logdet(I_576 + a V^T V) = logdet(I_96 + a V V^T); float64
    Cholesky on [16,96,96] Grams finishes the scalar loss.
"""

import numpy as np

_STATE = {}

# -------- fixed problem geometry (hardcoded per harness contract) --------
B, CCH, H, W = 16, 32, 192, 192
NCORES = 8
TPC = B // NCORES          # timesteps per core = 2
OUT = 24                   # pooled spatial size
PIX = OUT * OUT            # 576
M = 96                     # feature rows (3 maps x 32 channels)
ALPHA_E = 6.0              # 576 / (96 * eps)
ALPHA_C = 18.0             # 576 / (32 * eps)
PAD = 26                   # padded conv input edge
PPIX = PAD * PAD           # 676
NQ = 8                     # 24-row h-slabs per pass
HR = H // NQ               # 24 rows per slab


def _build_nc():
    import concourse.bass as bass
    import concourse.tile as tile
    from concourse import bacc, mybir

    DT = mybir.dt.float32
    BF = mybir.dt.bfloat16

    nc = bacc.Bacc(
        "TRN2", target_bir_lowering=False, debug=False, num_devices=NCORES
    )

    # x[g] for g = t*3+m : feature-map plane stacks, host-reordered
    x = nc.declare_dram_parameter("x", [TPC * 3, CCH, H, W], DT, isOutput=False)
    # block-diag conv weights: wt[(m,ic), (dy*3+dx)*96 + (m,oc)], bf16
    wt = nc.declare_dram_parameter("wt", [96, 9 * 96], BF, isOutput=False)
    # t1 weights at partitions matching their fmap: m1@0, m2@32, m0@96
    wtt1 = nc.declare_dram_parameter("wtt1", [128, 9 * 32], BF, isOutput=False)
    ident = nc.declare_dram_parameter("ident", [96, 96], BF, isOutput=False)
    g_out = nc.declare_dram_parameter("g_out", [TPC, M, M], DT, isOutput=True)

    with tile.TileContext(nc) as tc:
        with (
            tc.tile_pool(name="persist", bufs=1) as persist,
            tc.tile_pool(name="slabsA", bufs=3) as slabsA,
            tc.tile_pool(name="slabsB", bufs=3) as slabsB,
            tc.tile_pool(name="wsums", bufs=2) as wsumsA,
            tc.tile_pool(name="wsumsB", bufs=2) as wsumsB,
            tc.tile_pool(name="vt", bufs=2) as vtpool,
            tc.tile_pool(name="convps", bufs=2, space="PSUM") as convps,
            tc.tile_pool(name="vtps", bufs=2, space="PSUM") as vtps,
            tc.tile_pool(name="gramps", bufs=1, space="PSUM") as gramps,
        ):
            wt_sb = persist.tile([96, 9 * 96], BF, tag="wt")
            nc.gpsimd.dma_start(out=wt_sb[:], in_=wt.ap())
            wtt1_sb = persist.tile([128, 9 * 32], BF, tag="wtt1")
            nc.gpsimd.dma_start(out=wtt1_sb[:], in_=wtt1.ap())
            id_sb = persist.tile([96, 96], BF, tag="ident")
            nc.gpsimd.dma_start(out=id_sb[:], in_=ident.ap())

            # x-major reflect-padded pooled conv inputs (bf16)
            padA = persist.tile([128, PPIX], BF, tag="padA")  # planes 0-3
            padB = persist.tile([64, PPIX], BF, tag="padB")   # planes 4-5
            v_sb = persist.tile([96, TPC * PIX], BF, tag="v")
            g_sb = persist.tile([96, TPC * 96], DT, tag="g")

            pA3 = padA[:].rearrange("p (x y) -> p x y", x=PAD)
            pB3 = padB[:].rearrange("p (x y) -> p x y", x=PAD)

            # ---- pooling: interleaved A/B 24-row slabs ----
            for q in range(NQ):
                for gl, np_, pad3, spool, wpool in (
                    (0, 128, pA3, slabsA, wsumsA),
                    (4, 64, pB3, slabsB, wsumsB),
                ):
                    slab = spool.tile([np_, HR * W], DT, tag=f"slab{np_}")
                    nc.sync.dma_start(
                        out=slab[:],
                        in_=x.ap()[
                            gl : gl + np_ // 32, :, HR * q : HR * (q + 1), :
                        ].rearrange("g c h w -> (g c) (h w)"),
                    )
                    # stage 1: w-pool 8:1, contiguous inner reduce,
                    # transposed (x-major) destination
                    wsum = wpool.tile([np_, HR * OUT], DT, tag=f"ws{np_}")
                    nc.vector.tensor_reduce(
                        out=wsum[:].rearrange("p (x h) -> p h x", h=HR),
                        in_=slab[:].rearrange("p (h x b) -> p h x b", x=OUT, b=8),
                        axis=mybir.AxisListType.X,
                        op=mybir.AluOpType.add,
                    )
                    # stage 2: h-pool 8:1, contiguous inner reduce, bf16
                    # out into padded interior rows y = 3q+1 .. 3q+3
                    with nc.allow_low_precision(
                        reason="pooled conv input in bf16 (tol 2e-2)"
                    ):
                        nc.vector.tensor_reduce(
                            out=pad3[:np_, 1:25, 3 * q + 1 : 3 * q + 4],
                            in_=wsum[:].rearrange(
                                "p (x hg r) -> p x hg r", hg=HR // 8, r=8
                            ),
                            axis=mybir.AxisListType.X,
                            op=mybir.AluOpType.add,
                        )

            # ---- finish reflect pad: x-edge rows, then y-edge cols ----
            for pad3 in (pA3, pB3):
                nc.vector.tensor_copy(pad3[:, 0:1, 1:25], pad3[:, 2:3, 1:25])
                nc.vector.tensor_copy(pad3[:, 25:26, 1:25], pad3[:, 23:24, 1:25])
                nc.vector.tensor_copy(pad3[:, :, 0:1], pad3[:, :, 2:3])
                nc.vector.tensor_copy(pad3[:, :, 25:26], pad3[:, :, 23:24])

            dydx = [(a, b) for a in range(3) for b in range(3)]

            for t in range(TPC):
                # ---- conv: 9 shifted bf16 matmuls per x-half ----
                for xh in range(2):
                    pcb = convps.tile([96, 512], DT, tag="convps")
                    pc = pcb[:, 0:288]
                    if t == 0:
                        for i, (dy, dx) in enumerate(dydx):
                            blk = dy * 3 + dx
                            nc.tensor.matmul(
                                pc[:],
                                wt_sb[:, blk * 96 : (blk + 1) * 96],
                                pA3[:96, xh * 12 + dx : xh * 12 + dx + 12,
                                    dy : dy + 24],
                                start=(i == 0), stop=(i == 8),
                            )
                    else:
                        # t1 planes straddle padA[96:128] + padB
                        for m, (pad3, p0) in enumerate((
                            (pA3, 96), (pB3, 0), (pB3, 32),
                        )):
                            for i, (dy, dx) in enumerate(dydx):
                                blk = dy * 3 + dx
                                nc.tensor.matmul(
                                    pc[32 * m : 32 * m + 32, :],
                                    wtt1_sb[p0 : p0 + 32,
                                            blk * 32 : (blk + 1) * 32],
                                    pad3[p0 : p0 + 32,
                                         xh * 12 + dx : xh * 12 + dx + 12,
                                         dy : dy + 24],
                                    start=(i == 0), stop=(i == 8),
                                    tile_position=(p0 % 128, 32 * m),
                                )
                    # LeakyReLU(0.2) == max(0.2*z, z); PSUM feeds only one
                    # non-scalar input, so stage a copy through SBUF
                    zc = vtpool.tile([96, 288], DT, tag="zcopy")
                    nc.scalar.copy(zc[:], pc[:])
                    nc.vector.scalar_tensor_tensor(
                        out=v_sb[:, t * PIX + xh * 288 : t * PIX + (xh + 1) * 288],
                        in0=zc[:],
                        scalar=0.2,
                        in1=pc[:],
                        op0=mybir.AluOpType.mult,
                        op1=mybir.AluOpType.max,
                    )

                # ---- Gram: G_t += VT_chunk^T @ VT_chunk (bf16) ----
                gpb = gramps.tile([96, 512], DT, tag="gram")
                gp = gpb[:, 0:96]
                vt_all = vtpool.tile([128, 5 * 96], BF, tag="vtall")
                for c in range(5):
                    sz = 128 if c < 4 else 64
                    vslice = v_sb[:, t * PIX + c * 128 : t * PIX + c * 128 + sz]
                    ptb = vtps.tile([128, 1024], BF, tag="vtps")
                    pt = ptb[:, 0:96]
                    nc.tensor.transpose(pt[:sz, :], vslice, id_sb[:])
                    nc.scalar.copy(vt_all[:sz, c * 96 : (c + 1) * 96], pt[:sz, :])
                for c in range(5):
                    sz = 128 if c < 4 else 64
                    nc.tensor.matmul(
                        gp[:],
                        vt_all[:sz, c * 96 : (c + 1) * 96],
                        vt_all[:sz, c * 96 : (c + 1) * 96],
                        start=(c == 0), stop=(c == 4),
                    )
                nc.scalar.copy(g_sb[:, t * 96 : (t + 1) * 96], gp[:])
                nc.gpsimd.dma_start(
                    out=g_out[t], in_=g_sb[:, t * 96 : (t + 1) * 96]
                )

    nc.finalize()
    return nc


def _get_nc():
    if "nc" not in _STATE:
        _STATE["nc"] = _build_nc()
    return _STATE["nc"]


def _prep_weights(W1, W2, W3):
    import ml_dtypes

    # wt[(m,ic), (dy*3+dx)*96 + 32m+oc] = W_m[oc, ic, dy, dx] / 64
    wt = np.zeros((96, 9 * 96), dtype=np.float64)
    for m, Wm in enumerate((W1, W2, W3)):
        Wm = np.asarray(Wm, np.float64) / 64.0  # [oc, ic, dy, dx]
        for dy in range(3):
            for dx in range(3):
                blk = dy * 3 + dx
                wt[
                    32 * m : 32 * m + 32,
                    blk * 96 + 32 * m : blk * 96 + 32 * m + 32,
                ] = Wm[:, :, dy, dx].T
    # wtt1[p, blk*32+oc]: m0 block (W1) at rows 96-127, m1 (W2) at 0-31,
    # m2 (W3) at 32-63 -- partition-aligned with each t1 fmap
    wtt1 = np.zeros((128, 9 * 32), dtype=np.float64)
    for row, Wm in ((96, W1), (0, W2), (32, W3)):
        Wm = np.asarray(Wm, np.float64) / 64.0
        for dy in range(3):
            for dx in range(3):
                blk = dy * 3 + dx
                wtt1[row : row + 32, blk * 32 : (blk + 1) * 32] = Wm[:, :, dy, dx].T
    bf = ml_dtypes.bfloat16
    return wt.astype(np.float32).astype(bf), wtt1.astype(np.float32).astype(bf)


def _host_loss(G):
    G = np.asarray(G, np.float64)  # [16, 96, 96]
    T = G.shape[0]
    I96 = np.eye(M)
    Me = I96[None] + ALPHA_E * G
    ld_e = 2.0 * np.log(
        np.diagonal(np.linalg.cholesky(Me), axis1=-2, axis2=-1)
    ).sum()
    blocks = np.stack(
        [G[:, 32 * c : 32 * (c + 1), 32 * c : 32 * (c + 1)] for c in range(3)]
    )  # [3, T, 32, 32]
    Mc = np.eye(32)[None, None] + ALPHA_C * blocks
    ld_c = 2.0 * np.log(
        np.diagonal(np.linalg.cholesky(Mc), axis1=-2, axis2=-1)
    ).sum()
    loss_expd = ld_e / (2.0 * T)
    loss_comp = (32.0 / M) * ld_c / (2.0 * T)
    return np.float32(loss_expd - loss_comp)


def run_device(inputs, **kw):
    """Run the bass kernel; returns (G [16,96,96], BassKernelResults)."""
    import ml_dtypes
    from concourse.bass_utils import run_bass_kernel_spmd

    nc = _get_nc()
    wt, wtt1 = _prep_weights(inputs["W1"], inputs["W2"], inputs["W3"])
    ident = np.eye(96, dtype=np.float32).astype(ml_dtypes.bfloat16)
    ms = np.asarray(inputs["ms_fea"], np.float32)
    pan = np.asarray(inputs["pan_fea"], np.float32)
    alf = np.asarray(inputs["all_fea"], np.float32)
    in_maps = []
    for i in range(NCORES):
        sl = slice(TPC * i, TPC * (i + 1))
        # x[t*3+m] = (ms,pan,alf)[m][t]
        xs = np.stack([ms[sl], pan[sl], alf[sl]], axis=1).reshape(
            TPC * 3, CCH, H, W
        )
        in_maps.append(
            {"x": np.ascontiguousarray(xs), "wt": wt, "wtt1": wtt1,
             "ident": ident}
        )
    res = run_bass_kernel_spmd(nc, in_maps, core_ids=list(range(NCORES)), **kw)
    G = np.concatenate([np.asarray(r["g_out"]) for r in res.results], axis=0)
    return G, res


def kernel(**inputs):
    G, _ = run_device(inputs)
    return _host_loss(G)
